# revision 15
# baseline (speedup 1.0000x reference)
"""Trainium2 Bass kernel for nn_Attention_39487929319718.

Module: x:(4,128,64,64) -> 1x1conv QKV (+16 memory tokens) -> 4-head
attention (d=32) over n=4112 tokens -> drop memory queries -> 1x1 conv out.

Sharding: 8 cores = 4 batches x 2 query-halves. Each core receives its
batch's full x (token-rolled so its own 2048 queries sit first), computes
K/V for all 4112 tokens, attention for its 2048 queries across all 4 heads,
and the full output projection for those tokens. No collectives.

Per-core layout highlights:
 - Scores are computed transposed (k-tokens on partitions) so softmax'd
   probs feed the AV matmul directly with no transposes.
 - exp() has no max-subtraction (|scores| < 3 for this data distribution),
   with the 1/sqrt(d) scale folded into the activation's free affine.
 - V^T carries an extra ones column so each AV accumulation also produces
   the softmax denominator row for free.
"""

import sys

sys.path.insert(0, "/opt/trn_rl_repo")

import numpy as np

B, C, H, W = 4, 128, 64, 64
HEADS, DH, MEM = 4, 32, 16
HW = H * W            # 4096
NT = HW + MEM         # 4112 tokens (keys/values)
QN = HW // 2          # 2048 queries per core
SCALE = float(DH) ** -0.5

# k-token chunks of 128 (last chunk = 16 memory tokens)
KCHUNKS = [(j * 128, 128) for j in range(32)] + [(4096, 16)]
VTW = 33  # per-head vT columns per chunk: 32 v dims + 1 ones
VCW = HEADS * VTW  # 132 columns per chunk in the vT tile

_NC_CACHE = None


def _build_nc():
    import concourse.bacc as bacc
    import concourse.mybir as mybir
    import concourse.tile as tile

    F32 = mybir.dt.float32
    B16 = mybir.dt.bfloat16
    EXP = mybir.ActivationFunctionType.Exp

    nc = bacc.Bacc("TRN2", target_bir_lowering=False, debug=False, num_devices=8)

    xd = nc.dram_tensor("x", [C, HW], F32, kind="ExternalInput").ap()
    memd = nc.dram_tensor("mem", [C, MEM], F32, kind="ExternalInput").ap()
    wq_d = nc.dram_tensor("wqT", [C, 128], F32, kind="ExternalInput").ap()
    wk_d = nc.dram_tensor("wkT", [C, 128], F32, kind="ExternalInput").ap()
    wv_d = nc.dram_tensor("wvT", [C, 128], F32, kind="ExternalInput").ap()
    wo_d = nc.dram_tensor("woT", [128, C], F32, kind="ExternalInput").ap()
    bo_d = nc.dram_tensor("bout", [C, 1], F32, kind="ExternalInput").ap()
    outd = nc.dram_tensor("out", [C, QN], F32, kind="ExternalOutput").ap()

    with tile.TileContext(nc) as tc:
        with (
            tc.tile_pool(name="const", bufs=1) as constp,
            tc.tile_pool(name="big", bufs=1) as bigp,
            tc.tile_pool(name="pt", bufs=2) as ptp,
            tc.tile_pool(name="ep", bufs=2) as epp,
            tc.tile_pool(name="simps", bufs=2, space="PSUM") as simp,
            tc.tile_pool(name="avps", bufs=2, space="PSUM") as avp,
        ):
            # ---- constants / weights -------------------------------------
            wq_s = constp.tile([C, 128], F32, tag="wq")
            nc.sync.dma_start(out=wq_s[:], in_=wq_d)
            wk_s = constp.tile([C, 128], F32, tag="wk")
            nc.sync.dma_start(out=wk_s[:], in_=wk_d)
            wv_s = constp.tile([C, 128], F32, tag="wv")
            nc.sync.dma_start(out=wv_s[:], in_=wv_d)
            wo_s = constp.tile([128, C], F32, tag="wo")
            nc.sync.dma_start(out=wo_s[:], in_=wo_d)
            bo_s = constp.tile([C, 1], F32, tag="bo")
            nc.sync.dma_start(out=bo_s[:], in_=bo_d)

            # ---- x_ext = [x | memory] (chunked so projections start early)
            xe = bigp.tile([C, NT], F32, tag="xe")
            for i in range(8):
                nc.sync.dma_start(
                    out=xe[:, i * 512 : (i + 1) * 512], in_=xd[:, i * 512 : (i + 1) * 512]
                )
            nc.sync.dma_start(out=xe[:, HW:NT], in_=memd)

            # main-loop matmul operands in bf16 (accumulation stays fp32)
            Ksb = bigp.tile([C, NT], B16, tag="K")          # (4h x 32d, tok)
            Qsb = bigp.tile([C, QN], B16, tag="Q")          # (4h x 32d, q)
            VT = bigp.tile([C, 33, HEADS, VTW], B16, tag="VT")  # (tok%128, chunk, h, d+1)
            att = bigp.tile([C, QN], F32, tag="att")        # (4h x 32d, q) normalized
            osb = bigp.tile([C, QN], F32, tag="osb")

            # ---- projections ---------------------------------------------
            # Q: first 2048 columns (this core's queries);  K: all 4112 tokens
            for i in range(4):
                ps = avp.tile([128, 1024], F32, tag="avp", name=f"qps{i}")
                nc.tensor.matmul(
                    ps[:, 0:512], lhsT=wq_s[:], rhs=xe[:, i * 512 : (i + 1) * 512],
                    start=True, stop=True,
                )
                nc.vector.tensor_copy(Qsb[:, i * 512 : (i + 1) * 512], ps[:, 0:512])
            for off, nn in [(i * 512, 512) for i in range(8)] + [(HW, MEM)]:
                ps = avp.tile([128, 1024], F32, tag="avp", name=f"kps{off}")
                nc.tensor.matmul(
                    ps[:, 0:nn], lhsT=wk_s[:], rhs=xe[:, off : off + nn],
                    start=True, stop=True,
                )
                nc.vector.tensor_copy(Ksb[:, off : off + nn], ps[:, 0:nn])
            # vT: token-major V, one 128-token chunk at a time
            for j, (off, nn) in enumerate(KCHUNKS):
                ps = avp.tile([128, HEADS, 32], F32, tag="avp", name=f"vps{j}")
                nc.tensor.matmul(
                    ps[0:nn, :, :], lhsT=xe[:, off : off + nn], rhs=wv_s[:],
                    start=True, stop=True,
                )
                nc.vector.tensor_copy(VT[0:nn, j, :, 0:32], ps[0:nn, :, :])
            nc.vector.memset(VT[:, :, :, 32:33], 1.0)

            # ---- main attention loop -------------------------------------
            # Head-slab order: one (head, 1024-query block) slab at a time.
            # Per k-chunk: one N=1024 bf16 QK matmul -> one (128,1024) exp on
            # ACT (the kernel bottleneck) -> one N=1024 bf16 AV accumulation.
            for h in range(4):
                hl, hh = 32 * h, 32 * (h + 1)
                for qb in range(2):
                    qlo, qhi = qb * 1024, (qb + 1) * 1024
                    avt = avp.tile([128, 1024], F32, tag="avp", name=f"av{h}_{qb}")
                    for j, (off, nn) in enumerate(KCHUNKS):
                        sim = simp.tile([128, 1024], F32, tag="sim", name=f"s{h}_{qb}_{j}")
                        for u in range(2):
                            nc.tensor.matmul(
                                sim[0:nn, u * 512 : (u + 1) * 512],
                                lhsT=Ksb[hl:hh, off : off + nn],
                                rhs=Qsb[hl:hh, qlo + u * 512 : qlo + (u + 1) * 512],
                                start=True, stop=True,
                                tile_position=(hl, 0),
                            )
                        pt = ptp.tile([128, 1024], B16, tag="pt", name=f"p{h}_{qb}_{j}")
                        nc.scalar.activation(pt[0:nn, :], sim[0:nn, :], EXP, scale=SCALE)
                        for u in range(2):
                            nc.tensor.matmul(
                                avt[0:VTW, u * 512 : (u + 1) * 512],
                                lhsT=VT[0:nn, j, h, :],
                                rhs=pt[0:nn, u * 512 : (u + 1) * 512],
                                start=(j == 0), stop=(j == 32),
                            )

                    # normalize: att[32h+d, q] = avt[d, q] / avt[32, q]
                    dnm = epp.tile([1, 1024], F32, tag="dnm")
                    nc.vector.tensor_copy(dnm[:], avt[32:33, :])
                    rcp = epp.tile([1, 1024], F32, tag="rcp")
                    nc.vector.reciprocal_approx_fast(rcp[:], dnm[:])
                    rcp_rep = epp.tile([32, 1024], F32, tag="rcp_rep")
                    nc.gpsimd.partition_broadcast(rcp_rep[:], rcp[:])
                    nc.vector.tensor_mul(att[hl:hh, qlo:qhi], avt[0:32, :], rcp_rep[:])

            # ---- output projection + bias --------------------------------
            for qb in range(4):
                qlo, qhi = qb * 512, (qb + 1) * 512
                op = avp.tile([128, 1024], F32, tag="avp", name=f"op{qb}")
                nc.tensor.matmul(
                    op[:, 0:512], lhsT=wo_s[:], rhs=att[:, qlo:qhi],
                    start=True, stop=True,
                )
                nc.vector.tensor_scalar_add(osb[:, qlo:qhi], op[:, 0:512], bo_s[:])
                nc.sync.dma_start(out=outd[:, qlo:qhi], in_=osb[:, qlo:qhi])

    nc.compile()
    return nc


def get_nc():
    global _NC_CACHE
    if _NC_CACHE is None:
        _NC_CACHE = _build_nc()
    return _NC_CACHE


def make_in_maps(x, memory, w_qkv, w_out, b_out):
    """Host-side shard prep. Core c = 2*b + half handles batch b, queries
    [half*2048, half*2048+2048). Tokens are rolled so each core's queries
    occupy columns [0, 2048) -- attention is permutation-invariant in keys,
    so K/V over the rolled token set give identical outputs."""
    x = np.ascontiguousarray(np.asarray(x, dtype=np.float32).reshape(B, C, HW))
    mem = np.ascontiguousarray(np.asarray(memory, dtype=np.float32).reshape(C, MEM))
    w_qkv = np.asarray(w_qkv, dtype=np.float32)
    w_out = np.asarray(w_out, dtype=np.float32)
    b_out = np.asarray(b_out, dtype=np.float32)

    wqT = np.ascontiguousarray(w_qkv[0:128].T)
    wkT = np.ascontiguousarray(w_qkv[128:256].T)
    wvT = np.ascontiguousarray(w_qkv[256:384].T)
    woT = np.ascontiguousarray(w_out.T)
    bo = np.ascontiguousarray(b_out.reshape(C, 1))

    in_maps = []
    for core in range(8):
        b, half = divmod(core, 2)
        xb = x[b] if half == 0 else np.ascontiguousarray(np.roll(x[b], -QN, axis=1))
        in_maps.append(
            {
                "x": xb,
                "mem": mem,
                "wqT": wqT,
                "wkT": wkT,
                "wvT": wvT,
                "woT": woT,
                "bout": bo,
            }
        )
    return in_maps


def assemble(results):
    """results: list of 8 dicts with per-core 'out' of shape (C, QN)."""
    out = np.empty((B, C, HW), dtype=np.float32)
    for core in range(8):
        b, half = divmod(core, 2)
        out[b, :, half * QN : (half + 1) * QN] = results[core]["out"]
    return out.reshape(B, C, H, W)


def kernel(x, memory, w_qkv, w_out, b_out, _trace=False):
    from concourse.bass_utils import run_bass_kernel_spmd

    nc = get_nc()
    in_maps = make_in_maps(x, memory, w_qkv, w_out, b_out)
    res = run_bass_kernel_spmd(nc, in_maps, core_ids=list(range(8)), trace=_trace)
    out = assemble(res.results)
    if _trace:
        return out, res
    return out


# revision 18
# speedup vs baseline: 1.7547x; 1.7547x over previous
"""Trainium2 Bass kernel for nn_Attention_39487929319718.

Module: x:(4,128,64,64) -> 1x1conv QKV (+16 memory tokens) -> 4-head
attention (d=32) over n=4112 tokens -> drop memory queries -> 1x1 conv out.

Sharding: 8 cores = 4 batches x 2 query-halves. Each core receives its
batch's full x (token-rolled so its own 2048 queries sit first), computes
K/V for all 4112 tokens, attention for its 2048 queries across all 4 heads,
and the full output projection for those tokens. No collectives.

Per-core layout highlights:
 - Scores are computed transposed (k-tokens on partitions) so softmax'd
   probs feed the AV matmul directly with no transposes.
 - exp() has no max-subtraction (|scores| < 3 for this data distribution),
   with the 1/sqrt(d) scale folded into the activation's free affine.
 - V^T carries an extra ones column so each AV accumulation also produces
   the softmax denominator row for free.
"""

import sys

sys.path.insert(0, "/opt/trn_rl_repo")

import numpy as np

B, C, H, W = 4, 128, 64, 64
HEADS, DH, MEM = 4, 32, 16
HW = H * W            # 4096
NT = HW + MEM         # 4112 tokens (keys/values)
QN = HW // 2          # 2048 queries per core
SCALE = float(DH) ** -0.5

# k-token chunks of 128 (last chunk = 16 memory tokens)
KCHUNKS = [(j * 128, 128) for j in range(32)] + [(4096, 16)]
VTW = 33  # per-head vT columns per chunk: 32 v dims + 1 ones
VCW = HEADS * VTW  # 132 columns per chunk in the vT tile

_NC_CACHE = None


def _build_nc():
    import concourse.bacc as bacc
    import concourse.mybir as mybir
    import concourse.tile as tile

    F32 = mybir.dt.float32
    B16 = mybir.dt.bfloat16
    EXP = mybir.ActivationFunctionType.Exp

    nc = bacc.Bacc("TRN2", target_bir_lowering=False, debug=False, num_devices=8)

    xd = nc.dram_tensor("x", [C, HW], F32, kind="ExternalInput").ap()
    memd = nc.dram_tensor("mem", [C, MEM], F32, kind="ExternalInput").ap()
    wq_d = nc.dram_tensor("wqT", [C, 128], F32, kind="ExternalInput").ap()
    wk_d = nc.dram_tensor("wkT", [C, 128], F32, kind="ExternalInput").ap()
    wv_d = nc.dram_tensor("wvT", [C, 128], F32, kind="ExternalInput").ap()
    wo_d = nc.dram_tensor("woT", [128, C], F32, kind="ExternalInput").ap()
    bo_d = nc.dram_tensor("bout", [C, 1], F32, kind="ExternalInput").ap()
    outd = nc.dram_tensor("out", [C, QN], F32, kind="ExternalOutput").ap()

    with tile.TileContext(nc) as tc:
        with (
            tc.tile_pool(name="const", bufs=1) as constp,
            tc.tile_pool(name="big", bufs=1) as bigp,
            tc.tile_pool(name="pt", bufs=4) as ptp,
            tc.tile_pool(name="ep", bufs=3) as epp,
            tc.tile_pool(name="simps", bufs=3, space="PSUM") as simp,
            tc.tile_pool(name="avps", bufs=2, space="PSUM") as avp,
        ):
            # ---- constants / weights -------------------------------------
            wq_s = constp.tile([C, 128], F32, tag="wq")
            nc.sync.dma_start(out=wq_s[:], in_=wq_d)
            wk_s = constp.tile([C, 128], F32, tag="wk")
            nc.sync.dma_start(out=wk_s[:], in_=wk_d)
            wv_s = constp.tile([C, 128], F32, tag="wv")
            nc.sync.dma_start(out=wv_s[:], in_=wv_d)
            wo_s = constp.tile([128, C], F32, tag="wo")
            nc.sync.dma_start(out=wo_s[:], in_=wo_d)
            bo_s = constp.tile([C, 1], F32, tag="bo")
            nc.sync.dma_start(out=bo_s[:], in_=bo_d)

            # ---- x_ext = [x | memory] (chunked so projections start early)
            xe = bigp.tile([C, NT], F32, tag="xe")
            for i in range(8):
                nc.sync.dma_start(
                    out=xe[:, i * 512 : (i + 1) * 512], in_=xd[:, i * 512 : (i + 1) * 512]
                )
            nc.sync.dma_start(out=xe[:, HW:NT], in_=memd)

            # main-loop matmul operands in bf16 (accumulation stays fp32)
            Ksb = bigp.tile([C, NT], B16, tag="K")          # (4h x 32d, tok)
            Qsb = bigp.tile([C, QN], B16, tag="Q")          # (4h x 32d, q)
            VT = bigp.tile([C, 33, HEADS, VTW], B16, tag="VT")  # (tok%128, chunk, h, d+1)
            att = bigp.tile([C, QN], F32, tag="att")        # (4h x 32d, q) normalized
            osb = bigp.tile([C, QN], F32, tag="osb")

            # ---- projections ---------------------------------------------
            # Q: first 2048 columns (this core's queries);  K: all 4112 tokens
            for i in range(4):
                ps = avp.tile([128, 512], F32, tag="avp", name=f"qps{i}")
                nc.tensor.matmul(
                    ps[:, 0:512], lhsT=wq_s[:], rhs=xe[:, i * 512 : (i + 1) * 512],
                    start=True, stop=True,
                )
                nc.vector.tensor_copy(Qsb[:, i * 512 : (i + 1) * 512], ps[:, 0:512])
            for off, nn in [(i * 512, 512) for i in range(8)] + [(HW, MEM)]:
                ps = avp.tile([128, 512], F32, tag="avp", name=f"kps{off}")
                nc.tensor.matmul(
                    ps[:, 0:nn], lhsT=wk_s[:], rhs=xe[:, off : off + nn],
                    start=True, stop=True,
                )
                nc.vector.tensor_copy(Ksb[:, off : off + nn], ps[:, 0:nn])
            # vT: token-major V, one 128-token chunk at a time
            for j, (off, nn) in enumerate(KCHUNKS):
                ps = avp.tile([128, HEADS, 32], F32, tag="avp", name=f"vps{j}")
                nc.tensor.matmul(
                    ps[0:nn, :, :], lhsT=xe[:, off : off + nn], rhs=wv_s[:],
                    start=True, stop=True,
                )
                nc.vector.tensor_copy(VT[0:nn, j, :, 0:32], ps[0:nn, :, :])
            nc.vector.memset(VT[:, :, :, 32:33], 1.0)

            # ---- main attention loop -------------------------------------
            # Per k-chunk, head pair (2t, 2t+1) shares one (128,1024) score
            # tile; the two QK matmuls sit on distinct 32-row PE strips (they
            # share one XBUS stream of Qsb) and the two AV matmuls sit on
            # distinct col strips via tile_position (0,0)/(0,64), so each
            # pair runs concurrently on the array.
            for qb in range(4):
                qlo, qhi = qb * 512, (qb + 1) * 512
                av = [
                    avp.tile([128, 512], F32, tag="avp", name=f"av{qb}_{t}")
                    for t in range(2)
                ]
                for j, (off, nn) in enumerate(KCHUNKS):
                    for t in range(2):
                        sim = simp.tile([128, 1024], F32, tag="sim", name=f"s{qb}_{j}_{t}")
                        for s in range(2):
                            h = 2 * t + s
                            nc.tensor.matmul(
                                sim[0:nn, s * 512 : (s + 1) * 512],
                                lhsT=Ksb[32 * h : 32 * (h + 1), off : off + nn],
                                rhs=Qsb[32 * h : 32 * (h + 1), qlo:qhi],
                                start=True, stop=True,
                                tile_position=(32 * h, 0),
                            )
                        pt = ptp.tile([128, 1024], B16, tag="pt", name=f"p{qb}_{j}_{t}")
                        nc.scalar.activation(pt[0:nn, :], sim[0:nn, :], EXP, scale=SCALE)
                        for s in range(2):
                            h = 2 * t + s
                            nc.tensor.matmul(
                                av[t][64 * s : 64 * s + VTW, :],
                                lhsT=VT[0:nn, j, h, :],
                                rhs=pt[0:nn, s * 512 : (s + 1) * 512],
                                start=(j == 0), stop=(j == 32),
                                tile_position=(0, 64 * s),
                                skip_group_check=True,
                            )

                # normalize: att[32h+d, q] = av[d, q] / av[denom_row, q]
                for t in range(2):
                    for s in range(2):
                        h = 2 * t + s
                        lo = 64 * s
                        dnm = epp.tile([1, 512], F32, tag="dnm")
                        nc.vector.tensor_copy(dnm[:], av[t][lo + 32 : lo + 33, :])
                        rcp = epp.tile([1, 512], F32, tag="rcp")
                        nc.vector.reciprocal_approx_fast(rcp[:], dnm[:])
                        rcp_rep = epp.tile([32, 512], F32, tag="rcp_rep")
                        nc.gpsimd.partition_broadcast(rcp_rep[:], rcp[:])
                        nc.vector.tensor_mul(
                            att[32 * h : 32 * (h + 1), qlo:qhi],
                            av[t][lo : lo + 32, :],
                            rcp_rep[:],
                        )

                # output projection + bias for this query block
                op = avp.tile([128, 512], F32, tag="avp", name=f"op{qb}")
                nc.tensor.matmul(
                    op[:], lhsT=wo_s[:], rhs=att[:, qlo:qhi], start=True, stop=True
                )
                nc.vector.tensor_scalar_add(osb[:, qlo:qhi], op[:], bo_s[:])
                nc.sync.dma_start(out=outd[:, qlo:qhi], in_=osb[:, qlo:qhi])

    nc.compile()
    return nc


def get_nc():
    global _NC_CACHE
    if _NC_CACHE is None:
        _NC_CACHE = _build_nc()
    return _NC_CACHE


def make_in_maps(x, memory, w_qkv, w_out, b_out):
    """Host-side shard prep. Core c = 2*b + half handles batch b, queries
    [half*2048, half*2048+2048). Tokens are rolled so each core's queries
    occupy columns [0, 2048) -- attention is permutation-invariant in keys,
    so K/V over the rolled token set give identical outputs."""
    x = np.ascontiguousarray(np.asarray(x, dtype=np.float32).reshape(B, C, HW))
    mem = np.ascontiguousarray(np.asarray(memory, dtype=np.float32).reshape(C, MEM))
    w_qkv = np.asarray(w_qkv, dtype=np.float32)
    w_out = np.asarray(w_out, dtype=np.float32)
    b_out = np.asarray(b_out, dtype=np.float32)

    wqT = np.ascontiguousarray(w_qkv[0:128].T)
    wkT = np.ascontiguousarray(w_qkv[128:256].T)
    wvT = np.ascontiguousarray(w_qkv[256:384].T)
    woT = np.ascontiguousarray(w_out.T)
    bo = np.ascontiguousarray(b_out.reshape(C, 1))

    in_maps = []
    for core in range(8):
        b, half = divmod(core, 2)
        xb = x[b] if half == 0 else np.ascontiguousarray(np.roll(x[b], -QN, axis=1))
        in_maps.append(
            {
                "x": xb,
                "mem": mem,
                "wqT": wqT,
                "wkT": wkT,
                "wvT": wvT,
                "woT": woT,
                "bout": bo,
            }
        )
    return in_maps


def assemble(results):
    """results: list of 8 dicts with per-core 'out' of shape (C, QN)."""
    out = np.empty((B, C, HW), dtype=np.float32)
    for core in range(8):
        b, half = divmod(core, 2)
        out[b, :, half * QN : (half + 1) * QN] = results[core]["out"]
    return out.reshape(B, C, H, W)


def kernel(x, memory, w_qkv, w_out, b_out, _trace=False):
    from concourse.bass_utils import run_bass_kernel_spmd

    nc = get_nc()
    in_maps = make_in_maps(x, memory, w_qkv, w_out, b_out)
    res = run_bass_kernel_spmd(nc, in_maps, core_ids=list(range(8)), trace=_trace)
    out = assemble(res.results)
    if _trace:
        return out, res
    return out
